# revision 2
# baseline (speedup 1.0000x reference)
"""Trainium2 Bass kernel for the aperiodic real-space Ewald sum (N=4096).

Math: with w_ij = erf(d_ij/sqrt(2)) / (d_ij + eps) (symmetric),
    t_j   = sum_i q_i w_ij
    field = t/(2*pi) + 2*SELF_C*q
    pot   = (q . t)/(4*pi) + SELF_C*sum(q^2)

Layout: each core owns a 512-row block of OUTPUTS t_j; partitions carry j
(4 chunks of 128), the free axis carries all 4096 i (rotated by -c*512
per core so the diagonal block sits at local free offset ic*128). The
i-sum is a FREE-axis reduction fused into the last DVE op (accum_out) —
this avoids the gpsimd tensor_reduce(axis=C) partition-reduce of the
original design, which measured ~470us per call (~1.9ms of its 2.13ms).

Per 128-j chunk:
  PE : d^2 = s_j + s_i - 2 p_j.p_i as 8 bf16 K=19 matmuls into a
       full-PSUM [128,4096] tile. The 19 rows are a bf16 hi/lo split
       (12 cross-term rows, 3+3 s hi/mid/lo rows, one +EPS2 row) giving
       ~3e-3-exact d^2 at 1 cycle/row — 4x faster than fp32's 4-cycle
       mode, which measured ~76us total. A bf16 BIG*I matmul on the
       diagonal block makes the self-pair contribute ~2^-20*q instead of
       erf(0)/0 noise.
  ACT: g = Exp(-B_FIT * d^2)            [exp table]
       w = Abs_reciprocal_sqrt(m)       [abs_rsqrt table]
  DVE: m = A_FIT*g + d^2                (scalar_tensor_tensor, PSUM in1)
       z = w * q_bcast, accum_out -> tcol[:, ic]
  using the identity w(u) = erf(sqrt(u/2))/sqrt(u) == 1/sqrt(u + r(u))
  with r fitted as A_FIT*exp(-B_FIT*u) (max |dw| 2.1e-3) — two ACT
  passes instead of Sqrt+Abs_rsqrt+Erf, and NaN-safe for slightly
  negative d^2 noise (no Sqrt). w/q_bcast/z are bf16 (2x DVE rate);
  accumulation stays fp32.
tcol[p, ic] = t_{c*512+ic*128+p} is complete after its chunk, so one
[128,4] DMA per core at the end. q_bcast is built on-device in the
preamble (ones-outer-product matmul + ACT copy). Host assembles
pot/field from the gathered t in float64.

Measured on the axon trn2 runtime: ~40-55us per iteration vs 2128us for
the original gpsimd-reduce kernel (~40-50x), rel err ~6e-4 (gate 2e-2).
"""
import sys

sys.path.insert(0, "/opt/trn_rl_repo")

import numpy as np
import ml_dtypes

import concourse.bass as bass
import concourse.tile as tile
from concourse import bacc, mybir
from concourse.bass_utils import run_bass_kernel_spmd

N = 4096
NCORES = 8
R = N // NCORES          # j-rows per core
CH = R // 128            # 128-j chunks per core
SIGMA = 1.0
TWOPI = 2.0 * np.pi
SELF_C = 1.0 / (SIGMA * TWOPI**1.5)
BIG = float(2.0**40)
EPS2 = 0.01
# w(u) = erf(sqrt(u)/sqrt(2))/sqrt(u) == 1/sqrt(u + r(u)); r fitted as
# A_FIT*exp(-B_FIT*u) over u in [1e-3, 1500]: max |dw| = 2.1e-3.
A_FIT = 1.573370
B_FIT = 0.327196

_nc_cache = None


def _build_nc(loop_n=None, unroll=1):
    nc = bacc.Bacc("TRN2", target_bir_lowering=False, debug=False,
                   num_devices=NCORES)
    f32 = mybir.dt.float32
    bf16 = mybir.dt.bfloat16
    E = mybir.ActivationFunctionType
    mult = mybir.AluOpType.mult
    add = mybir.AluOpType.add
    KR = 19

    aug_d = nc.dram_tensor("aug", [KR, R + N], bf16, kind="ExternalInput").ap()
    qrow_d = nc.dram_tensor("qrow", [1, N], f32, kind="ExternalInput").ap()
    ones_d = nc.dram_tensor("ones", [1, 128], f32, kind="ExternalInput").ap()
    idn_d = nc.dram_tensor("idn", [128, 256], bf16, kind="ExternalInput").ap()
    tcol_d = nc.dram_tensor("tcol", [128, CH], f32, kind="ExternalOutput").ap()

    with tile.TileContext(nc) as tc:
        with (
            tc.tile_pool(name="sbin", bufs=1) as sbin,
            tc.tile_pool(name="psum", bufs=1, space="PSUM") as psum,
            tc.tile_pool(name="sbd", bufs=2) as sbd,
            tc.tile_pool(name="sbr", bufs=2) as sbr,
            tc.tile_pool(name="sbc", bufs=2) as sbc,
            tc.tile_pool(name="sbz", bufs=1) as sbz,
        ):
            aug = sbin.tile([KR, R + N], bf16, tag="aug")
            nc.sync.dma_start(aug[:], aug_d[:])
            qrow = sbin.tile([1, N], f32, tag="qrow")
            nc.sync.dma_start(qrow[:], qrow_d[:])
            ones = sbin.tile([1, 128], f32, tag="ones")
            nc.sync.dma_start(ones[:], ones_d[:])
            idn = sbin.tile([128, 256], bf16, tag="idn")
            nc.sync.dma_start(idn[:], idn_d[:])

            # q broadcast [128, N] in bf16: ones[1,128].T @ qrow[1,N] via
            # PE, then copy PSUM -> SBUF. Same psum tag as the main loop so
            # the pool reuses the single full-PSUM buffer.
            qb = sbin.tile([128, N], bf16, tag="qb")
            pq = psum.tile([128, N], f32, tag="p")
            for k4 in range(N // 512):
                nc.tensor.matmul(
                    pq[:, k4 * 512:(k4 + 1) * 512],
                    ones[:], qrow[:, k4 * 512:(k4 + 1) * 512],
                    start=True, stop=True)
            nc.scalar.activation(qb[:], pq[:], E.Copy)

            tcol = sbz.tile([128, CH], f32, tag="tcol")

            def body():
                for ic in range(CH):
                    p = psum.tile([128, N], f32, tag="p")
                    for k4 in range(N // 512):
                        nc.tensor.matmul(
                            p[:, k4 * 512:(k4 + 1) * 512],
                            aug[:, ic * 128:(ic + 1) * 128],
                            aug[:, R + k4 * 512:R + (k4 + 1) * 512],
                            start=True, stop=not (k4 == 0))
                        if k4 == 0:
                            nc.tensor.matmul(
                                p[:, ic * 128:(ic + 1) * 128],
                                idn[:, 0:128], idn[:, 128:256],
                                start=False, stop=True)
                    g = sbd.tile([128, N], f32, tag="g")
                    nc.scalar.activation(g[:], p[:], E.Exp, scale=-B_FIT)
                    m = sbr.tile([128, N], f32, tag="m")
                    nc.vector.scalar_tensor_tensor(
                        out=m[:], in0=g[:], scalar=A_FIT, in1=p[:],
                        op0=mult, op1=add)
                    w = sbc.tile([128, N], bf16, tag="w")
                    nc.scalar.activation(w[:], m[:], E.Abs_reciprocal_sqrt)
                    z = sbz.tile([128, N], bf16, tag="z")
                    nc.vector.scalar_tensor_tensor(
                        out=z[:], in0=w[:], scalar=1.0, in1=qb[:],
                        op0=mult, op1=mult,
                        accum_out=tcol[:, ic:ic + 1])

            if loop_n is not None:
                with tc.For_i(0, loop_n, 1):
                    for _ in range(unroll):
                        body()
            else:
                body()

            nc.sync.dma_start(tcol_d[:], tcol[:])
    nc.compile()
    return nc


def _hilo(v, dt):
    hi = v.astype(dt)
    lo = (v - hi.astype(np.float32)).astype(dt)
    return hi, lo


def _himl(v, dt):
    hi = v.astype(dt)
    mid = (v - hi.astype(np.float32)).astype(dt)
    lo = (v - hi.astype(np.float32) - mid.astype(np.float32)).astype(dt)
    return hi, mid, lo


def _prep_inputs(positions, q):
    pos = np.ascontiguousarray(np.asarray(positions, dtype=np.float32))
    qv = np.asarray(q, dtype=np.float32).reshape(-1)
    s = (pos * pos).sum(axis=1, dtype=np.float32)

    idn = np.concatenate([np.eye(128), np.eye(128) * BIG],
                         axis=1).astype(ml_dtypes.bfloat16)
    ones = np.ones((1, 128), np.float32)

    # 19-row bf16 hi/lo split: d^2 exact to ~3e-3 abs at bf16-matmul
    # speed (matmul time is K-independent). Row pairing (lhs j | rhs i):
    #   3 coords x {hi,lo}x{hi,lo} = 12 rows of the -2 p_j.p_i cross
    #   term, s_j{hi,mid,lo}|1, 1|s_i{hi,mid,lo}, and a constant EPS2|1
    #   row so d^2 >= EPS2 - err > 0 (the closest pair in the data has
    #   d^2 ~ 2e-6, below the split error; the +EPS2 shift moves every
    #   w by <~1e-3 while keeping the exp/rsqrt inputs well-behaved).
    b16 = ml_dtypes.bfloat16
    lhs_all = np.empty((19, N), b16)
    rhs_all = np.empty((19, N), b16)
    for x in range(3):
        chi, clo = _hilo(pos[:, x], b16)
        mhi, mlo = _hilo(-2.0 * pos[:, x], b16)
        r = 4 * x
        lhs_all[r + 0], rhs_all[r + 0] = mhi, chi
        lhs_all[r + 1], rhs_all[r + 1] = mhi, clo
        lhs_all[r + 2], rhs_all[r + 2] = mlo, chi
        lhs_all[r + 3], rhs_all[r + 3] = mlo, clo
    sj = _himl(s, b16)
    one = b16(1.0)
    for k in range(3):
        lhs_all[12 + k], rhs_all[12 + k] = sj[k], one
        lhs_all[15 + k], rhs_all[15 + k] = one, sj[k]
    lhs_all[18], rhs_all[18] = b16(EPS2), one

    in_maps = []
    for c in range(NCORES):
        blk = slice(c * R, (c + 1) * R)
        aug = np.empty((19, R + N), b16)
        aug[:, 0:R] = lhs_all[:, blk]
        aug[:, R:] = np.roll(rhs_all, -c * R, axis=1)
        qrow = np.roll(qv, -c * R).reshape(1, N)
        in_maps.append({"aug": aug, "qrow": qrow, "ones": ones, "idn": idn})
    return in_maps, qv


def kernel(positions, q):
    global _nc_cache
    if _nc_cache is None:
        _nc_cache = _build_nc()
    nc = _nc_cache

    in_maps, qv = _prep_inputs(positions, q)
    res = run_bass_kernel_spmd(nc, in_maps, core_ids=list(range(NCORES)))

    t = np.empty(N, np.float64)
    for c in range(NCORES):
        tc_ = res.results[c]["tcol"].astype(np.float64)  # [128, CH]
        t[c * R:(c + 1) * R] = tc_.T.reshape(R)

    q64 = qv.astype(np.float64)
    field = t / TWOPI + 2.0 * SELF_C * q64
    pot = float((q64 * t).sum() / (2.0 * TWOPI) + SELF_C * (q64 * q64).sum())
    out = np.empty(N + 1, np.float32)
    out[0] = pot
    out[1:] = field.astype(np.float32)
    return out
